# revision 46
# baseline (speedup 1.0000x reference)
"""Multi-head self-attention (B=2, N=4096, D=512, H=8, dh=64) on 8 trn2 cores.

Sharding: batch*heads across cores. Core c handles batch b=c//4 and the
head-pair p=c%4 (a 128-wide slice of the inner dim). Each core computes its
two heads' attention and a partial output projection; the host sums the 4
partials per batch and adds the bias.

Structure (the Tile scheduler reorders by deps; rings/pools encode the
pipeline). ~220us on the CoreSim cost model vs the 300us v1 baseline; PE,
ScalarE and VectorE all sit at 84-88% busy:
 - scores ST[j, i] (kT slab stationary, qT moving, bf16). The exp of each
   [128, 1024] block is split COLUMN-WISE: ScalarE runs activation-Exp on
   [0:sc) (scale=16 on SCALE/16-pre-scaled scores) while VectorE runs the
   custom DVE poly op EXP_POLY16_ANT on [sc:1024) -- the halves execute
   concurrently, so the per-step exp latency that paces the loop through
   the 3-deep st PSUM ring is the max of the halves, not the sum.
 - AV is FLIPPED: out O[i, d] accumulates with the attention-weight tile as
   the STATIONARY operand (lhsT = et[:, i-block], 128x128; the cost model
   does not charge stationary loads) and V (65 cols, ones column at 64 for
   the softmax denominator) as the moving operand: 65 rows per (i-block, j)
   instead of 512 per (j, chunk) -- AV drops from 262k to 133k PE cycles.
   AV for (w, h) is smoothed into the NEXT head's score loop at
   av_per_step matmuls per step so PE load stays level.
 - softmax normalize: O[i, 64] is the per-partition denominator, so the
   whole normalize is one gpsimd normalize_recip per i-block, written
   straight into a both-heads collect buffer [i, (h, d)]; one 128x128 DMA
   XBAR transpose per i-block then writes the full OTn column block.
   (The XBAR transpose corrupts STRIDED destinations on real hardware --
   OTn's contiguous column blocks are fine, V2's strided [*, jb, h, d]
   layout is not, so V2 is still built with PE transposes.)
 - x, Wq/k/v are loaded as bf16 (halves DMA; adds ~2e-3 rel err, budget
   2e-2), Wo/OTn are fp16; the projections for windows 2-3 (x blocks 4-7)
   are deferred out of the PE-bound (0,0) loop into later exp-paced loops.
 - a ~2.6us PE warm-up on const data before x lands beats the 0.65->2.4GHz
   p-state ramp; the output projection of window w-1 splits 4/4 across the
   two following loops; the last loop carries two of its own AV i-blocks
   in-loop so the drain tail is ~5us.
 - HW-path race caution: CoreSim tolerates emission orders that race on
   real hardware. Two observed on this kernel: (1) reusing an SBUF ring
   buffer via a DMA while earlier-emitted engine reads are in flight
   (x_bufs=8 sidesteps it -- the 8 x blocks never recycle); (2) moving the
   first v-projections out of the prologue into the fill flips the final
   output to garbage even with no ring reuse. Keep the 6 prologue
   projection units and verify any emission-order change on hardware, not
   just in CoreSim.
"""

from contextlib import ExitStack

import numpy as np

HEADS = 8
DIM_HEAD = 64
SCALE = DIM_HEAD ** -0.5
B, N, D = 2, 4096, 512
NCORES = 8
E = 128          # inner-dim slice per core (2 heads)
JB = 128         # key block (contraction partition dim)
WI = 1024        # query i-window (et tile width)
SV = 68          # V free-dim stride per j-block (65 used)

_CACHE = {}


def _exp_poly_op():
    """Register (once) the custom DVE op EXP_POLY16_ANT:
    out = (1 + w + w^2*s0)^16 with w = in0 (pre-scaled scores), s0 = 1/2.
    This is exp(16*w) to ~7e-4 relative for |16*w| <= 1 (the score range);
    qT is pre-scaled by SCALE/16 so w = s*SCALE/16."""
    if "exp_op" in _CACHE:
        return _CACHE["exp_op"]
    import concourse.dve_ops as dve_ops
    from concourse.dve_ops import DveOp, OPS
    from concourse.dve_spec import Spec, Src0, C0, One, lower, sq
    from concourse.dve_spec import _has_src1
    from concourse.dve_uop import DveOpSpec

    name = "EXP_POLY16_ANT"
    if name not in dve_ops._SUB_OPCODE_FOR_NAME:
        p = (sq(Src0) * C0) + (Src0 + One)
        body = sq(sq(sq(sq(p))))
        spec = Spec(
            body=body,
            reference=lambda in0, in1, s0, s1, imm2: (
                1.0 + in0 + in0 * in0 * s0
            ) ** 16,
        )
        row = dve_ops._CUSTOM_DVE_ROW_BASE + len(OPS)
        assert row < 0x20, "custom-DVE row field overflow"
        shas = {}
        for ver in ("v3", "v4"):
            s_obj = DveOpSpec(name=name, opcode=row, uops=lower(spec, ver=ver),
                              rd1_en=_has_src1(spec))
            shas[ver] = s_obj.sha(ver)
        op = DveOp(name, spec, subdim=False, uops_sha=shas)
        OPS.append(op)
        dve_ops.CUSTOM_DVE_SPECS[name] = spec
        dve_ops._SUB_OPCODE_FOR_NAME[name] = row
    _CACHE["exp_op"] = next(o for o in OPS if o.name == name)
    return _CACHE["exp_op"]


# which j-blocks (mod 16) run exp on VectorE (custom DVE op) vs ScalarE
DVE_PAT = (0, 1, 0, 0, 1, 0, 0, 1, 0, 0, 1, 0, 0, 1, 0, 0)
DVE_PAT0 = (0, 0, 0, 0, 1, 0, 0, 0, 0, 0, 0, 0, 1, 0, 0, 0)


def build_program(n=N, wi_=None, st_bufs=3, o_bufs=2, pt_bufs=2, et_bufs=50,
                  x_bufs=8, av_per_step=13, sc=604, sc0=768, wu_n=3,
                  op_j0=8, op_dj=3, use_custom=1, norm_gpsimd=1, dbg=0,
                  sc_map={(0, 1): 544}, op_j0h0=13):
    import concourse.bass as bass
    import concourse.tile as tile
    from concourse import bacc, mybir
    from concourse.masks import make_identity

    f32 = mybir.dt.float32
    f32r = mybir.dt.float32r
    bf16 = mybir.dt.bfloat16
    f16 = mybir.dt.float16
    Exp = mybir.ActivationFunctionType.Exp
    exp_op = _exp_poly_op() if use_custom else None

    wi = wi_ or WI
    nj = n // JB             # 128-key blocks
    nw = max(1, n // wi)
    nwc = wi // 512          # 512-chunks per window
    nib = wi // 128          # 128-row i-blocks per window
    nnb = n // 512           # 512-blocks over full seq

    nc = bacc.Bacc("TRN2", target_bir_lowering=False, debug=False,
                   num_devices=NCORES)

    xT = nc.dram_tensor("xT", [D, n], bf16, kind="ExternalInput").ap()
    wqT = nc.dram_tensor("wqT", [D, E], bf16, kind="ExternalInput").ap()
    wkT = nc.dram_tensor("wkT", [D, E], bf16, kind="ExternalInput").ap()
    wvT = nc.dram_tensor("wvT", [D, E], bf16, kind="ExternalInput").ap()
    woT = nc.dram_tensor("woT", [E, D], f16, kind="ExternalInput").ap()
    yT = nc.dram_tensor("yT", [D, n], f32, kind="ExternalOutput").ap()
    if dbg:
        dbg_qT = nc.dram_tensor("dbg_qT", [E, n], mybir.dt.bfloat16,
                                kind="ExternalOutput").ap()
        dbg_kT = nc.dram_tensor("dbg_kT", [E, n], mybir.dt.bfloat16,
                                kind="ExternalOutput").ap()
        dbg_V2 = nc.dram_tensor("dbg_V2", [JB, nj * 2 * SV], mybir.dt.float16,
                                kind="ExternalOutput").ap()
        dbg_OTn = nc.dram_tensor("dbg_OTn", [E, n], mybir.dt.float16,
                                 kind="ExternalOutput").ap()

    def emit_body(tc, ctx):
        const = ctx.enter_context(tc.tile_pool(name="const", bufs=1))
        persist = ctx.enter_context(tc.tile_pool(name="persist", bufs=1))
        xp = ctx.enter_context(tc.tile_pool(name="xp", bufs=x_bufs))
        etp = ctx.enter_context(tc.tile_pool(name="etp", bufs=et_bufs))
        psA = ctx.enter_context(
            tc.tile_pool(name="psA", bufs=st_bufs, space="PSUM"))
        psO = ctx.enter_context(
            tc.tile_pool(name="psO", bufs=o_bufs, space="PSUM"))
        osbp = ctx.enter_context(tc.tile_pool(name="osbp", bufs=4))
        collp = ctx.enter_context(tc.tile_pool(name="collp", bufs=18))
        ysbp = ctx.enter_context(tc.tile_pool(name="ysb", bufs=6))

        identb = const.tile([128, 128], f16, name="identb", tag="identb")
        make_identity(nc, identb)

        # persistent SBUF tensors
        qT = persist.tile([E, n], bf16, name="qT", tag="qT")
        kT = persist.tile([E, n], bf16, name="kT", tag="kT")
        vT = persist.tile([E, n], f16, name="vT", tag="vT")
        OTn = persist.tile([E, n], f16, name="OTn", tag="OTn")
        V2 = persist.tile([JB, nj, 2, SV], f16, name="V2", tag="V2")
        wo_sb = persist.tile([E, D], f16, name="wo_sb", tag="wo_sb")

        # weights on the ACT DGE ring so they don't delay x on the SP ring
        wq_sb = persist.tile([128, 4, E], bf16, name="wq_sb", tag="wq_sb")
        wk_sb = persist.tile([128, 4, E], bf16, name="wk_sb", tag="wk_sb")
        wv_sb = persist.tile([128, 4, E], bf16, name="wv_sb", tag="wv_sb")
        for wsb, wdram in ((wq_sb, wqT), (wk_sb, wkT), (wv_sb, wvT)):
            nc.scalar.dma_start(
                out=wsb, in_=wdram.rearrange("(kc p) e -> p kc e", kc=4))
        nc.scalar.dma_start(out=wo_sb, in_=woT)

        # ones column of V (disjoint region from the data columns)
        nc.vector.memset(V2[:, :, :, DIM_HEAD:DIM_HEAD + 1], 1.0)

        # PE p-state warm-up: the tensor engine ramps 0.65->1.2->2.4GHz over
        # ~3us of continuous work. Chew on const data while the first x/w
        # DMAs are in flight so the projections start at full clock.
        if wu_n:
            wu = const.tile([128, 512], bf16, name="wu", tag="wu")
            nc.vector.memset(wu, 0.0)
            wups = psA.tile([128, wi], f32, name="wups", tag="st")
            for i in range(wu_n):
                nc.tensor.matmul(
                    wups[:, (i % 2) * 512:(i % 2) * 512 + 512],
                    lhsT=wu[:, 0:128], rhs=wu,
                    start=True, stop=True)

        x_tiles = {}

        def x_dma(nb):
            t = xp.tile([128, 4, 512], bf16, name="xt", tag="xt")
            x_tiles[nb] = t
            # one DMA per block (the ~565ns sequencer cost per instruction
            # adds up); alternate the sync/vector rings to halve per-ring
            # queueing; src AP views xT's [4*128, 512] slice as [p, kc, i]
            # first blocks must not queue behind the weight DMAs (Act ring)
            eng = nc.sync if (nb < 4 or nb % 2 == 0) else nc.scalar
            eng.dma_start(
                out=t,
                in_=xT[:, nb * 512:(nb + 1) * 512].rearrange(
                    "(kc p) i -> p kc i", kc=4))

        def proj_sub(nb, which):
            """One projection (q, k or v) for one 512-wide block."""
            sl = slice(nb * 512, (nb + 1) * 512)
            wsb, dest = {"q": (wq_sb, qT), "k": (wk_sb, kT),
                         "v": (wv_sb, vT)}[which]
            ps = psO.tile([128, 512], f32, name="pp", tag="po")
            for kc in range(4):
                nc.tensor.matmul(
                    ps,
                    lhsT=wsb[:, kc, :],
                    rhs=x_tiles[nb][:, kc, :],
                    start=(kc == 0), stop=(kc == 3))
            if which == "q":
                # scores arrive pre-scaled by SCALE/16 (ScalarE exp uses
                # scale=16, the DVE poly needs no input multiply)
                nc.vector.tensor_scalar(
                    out=dest[:, sl], in0=ps,
                    scalar1=float(SCALE / 16.0), scalar2=None,
                    op0=mybir.AluOpType.mult)
            else:
                nc.vector.tensor_copy(dest[:, sl], ps)

        def proj_unit(nb):
            for which in ("q", "k", "v"):
                proj_sub(nb, which)

        def trans_unit(nb):
            """V natural fp16 layout for this block's 4 key blocks: four PE
            transposes into one PSUM tile, one strided copy per head.
            (The DMA XBAR transpose corrupts strided destinations on real
            hardware, so V2 is built on the PE.)"""
            tpt = psO.tile([128, 1024], f16, name="tpt", tag="po")
            for jj in range(4):
                jb = nb * 4 + jj
                nc.tensor.transpose(tpt[:, jj * 128:(jj + 1) * 128],
                                    vT[:, jb * 128:(jb + 1) * 128], identb)
            src = tpt[:, 0:512].rearrange("p (jj h d) -> p jj h d", jj=4, h=2)
            for h in range(2):
                nc.vector.tensor_copy(
                    V2[:, nb * 4:(nb + 1) * 4, h, 0:DIM_HEAD],
                    src[:, :, h:h + 1, :])

        def score_exp(w, h, j):
            """Score matmuls + exp for one key block; returns et [j, i].
            The exp is split column-wise: ScalarE does [0:sc_use), the DVE
            custom poly op does [sc_use:wi) -- the two halves run
            CONCURRENTLY, so the per-step exp latency (which paces the whole
            loop through the st ring) is the max of the halves, not the sum."""
            e0, e1 = h * 64, (h + 1) * 64
            et = etp.tile([128, wi], f16, name="et", tag="et")
            st = psA.tile([128, wi], f32, name="st", tag="st")
            for c2 in range(nwc):
                i0 = w * wi + c2 * 512
                nc.tensor.matmul(
                    st[:, c2 * 512:(c2 + 1) * 512],
                    lhsT=kT[e0:e1, j * JB:(j + 1) * JB],
                    rhs=qT[e0:e1, i0:i0 + 512],
                    start=True, stop=True)
            if w == 0 and h == 0:
                sc_use = sc0
            else:
                sc_use = (sc_map or {}).get((w, h), sc)
            if exp_op is None or sc_use >= wi:
                nc.scalar.activation(et, st, Exp, scale=16.0)
            else:
                nc.scalar.activation(et[:, 0:sc_use], st[:, 0:sc_use],
                                     Exp, scale=16.0)
                nc.vector._custom_dve(exp_op, out=et[:, sc_use:wi],
                                      in0=st[:, sc_use:wi], s0=0.5)
            return et

        COLL = {}

        def o_finish(hp, wp, ib, O):
            """Evacuate + normalize one finished O i-block (DVE + gpsimd)
            into the (w, ib) collect buffer [i, (h, d)]; once both heads have
            landed, ONE 128x128 DMA XBAR transpose writes the full OTn
            column block (no PE transposes, no DVE copies)."""
            osb = osbp.tile([128, 66], f32, name="osb", tag="osb")
            nc.vector.tensor_copy(osb[:, 0:DIM_HEAD + 1], O[:, 0:DIM_HEAD + 1])
            if hp == 0:
                COLL[(wp, ib)] = collp.tile([128, 2, DIM_HEAD], f16,
                                            name="coll", tag="coll")
            coll = COLL[(wp, ib)]
            nc.gpsimd.normalize_recip(
                out_ap=coll[:, hp, :], in_ap=osb[:, 0:DIM_HEAD],
                denom_ap=osb[:, DIM_HEAD:DIM_HEAD + 1])

        def otn_dma(wp, ib):
            """Deferred OTn column-block write (both heads landed)."""
            nc.sync.dma_start_transpose(
                out=OTn[:, wp * wi + ib * 128:wp * wi + (ib + 1) * 128],
                in_=COLL.pop((wp, ib)))

        def outproj_unit(w, k, tail=False):
            """One of the 8 output-projection blocks of window w."""
            ib = w * nwc + k // 4
            dc = k % 4
            ps2 = psO.tile([128, 512], f32, name="ps2", tag="po")
            nc.tensor.matmul(
                ps2,
                lhsT=wo_sb[:, dc * 128:(dc + 1) * 128],
                rhs=OTn[:, ib * 512:(ib + 1) * 512],
                start=True, stop=True)
            yt = ysbp.tile([128, 512], f32, name="yt", tag="yt")
            # in the tail ScalarE is idle; split the evacuation
            if tail and k % 2 == 0:
                nc.scalar.copy(yt, ps2)
            else:
                nc.vector.tensor_copy(yt, ps2)
            eng = nc.scalar if (tail and k % 2 == 1) else nc.sync
            eng.dma_start(
                out=yT[dc * 128:(dc + 1) * 128,
                       ib * 512:(ib + 1) * 512],
                in_=yt)

        ETS = {}

        def make_av_fill(hp, wp, outproj_w=None, op_base=0, op_cnt=8,
                         op_j0_=None, extra=None, extra_j=26, tail_h=None):
            """fill(j) that interleaves AV+finish of (hp, wp) and optionally
            the output projection of window outproj_w."""
            ets = ETS.pop((wp, hp))
            O_state = {}
            g = [0]
            done_op = [0]
            pending_dma = []
            pending_f = []
            tail_done = [0]

            def fill(j, cur_ets=None):
                # finish groups one step late so the DVE-side evacuation
                # never head-blocks the exp stream while waiting on the
                # group's stop matmul
                if pending_f:
                    ib, O = pending_f.pop(0)
                    o_finish(hp, wp, ib, O)
                    if hp == 1:
                        pending_dma.append(ib)
                take = min(av_per_step, nib * nj - g[0])
                for _ in range(take):
                    ib, jj = divmod(g[0], nj)
                    if jj == 0:
                        O_state["O"] = psO.tile([128, 512], f32,
                                                name="O", tag="po")
                    nc.tensor.matmul(
                        O_state["O"][:, 0:DIM_HEAD + 1],
                        lhsT=ets[jj][:, ib * 128:(ib + 1) * 128],
                        rhs=V2[:, jj, hp, 0:DIM_HEAD + 1],
                        start=(jj == 0), stop=(jj == nj - 1))
                    g[0] += 1
                    if jj == nj - 1:
                        pending_f.append((ib, O_state["O"]))
                if pending_dma and (g[0] >= (pending_dma[0] + 2) * nj
                                    or j >= nj - 2):
                    otn_dma(wp, pending_dma.pop(0))
                if extra is not None and j == extra_j:
                    extra()
                # last loop: its OWN first two AV i-blocks accumulate
                # in-loop (allocated after every other po-ring user so the
                # ring order stays acyclic); the tail then starts at ib=2
                if tail_h is not None and j >= 20:
                    if not tail_O:
                        for ib in range(2):
                            tail_O[ib] = psO.tile([128, 512], f32,
                                                  name="tO", tag="po")
                    hi = min(j - 1, nj - 1)
                    while tail_rem[0] <= hi:
                        jj = tail_rem[0]
                        for ib in range(2):
                            tail_av_mm(tail_O[ib], ib, cur_ets[jj], jj,
                                       tail_h)
                        tail_rem[0] += 1
                j0 = op_j0 if op_j0_ is None else op_j0_
                if (outproj_w is not None and j >= j0
                        and (j - j0) % op_dj == 0 and done_op[0] < op_cnt):
                    outproj_unit(outproj_w, op_base + done_op[0])
                    done_op[0] += 1

            return fill

        def head_loop(w, h, fill):
            ets = []
            for j in range(nj):
                ets.append(score_exp(w, h, j))
                fill(j, ets)
            ETS[(w, h)] = ets

        # ---- (0, 0): projections + V transposes fill the score loop ----
        for nb in range(4):
            x_dma(nb)
        for which in ("q", "k", "v"):
            proj_sub(0, which)
            proj_sub(1, which)

        def fill_w0h0(j, cur_ets=None):
            nb = j // 4 + 2
            if j % 4 == 0 and j // 4 + 4 < nnb:
                x_dma(j // 4 + 4)
            if j % 4 == 1 and nb <= 3:
                proj_sub(nb, "q")
            elif j % 4 == 2 and nb < nnb:
                proj_sub(nb, "k")
            elif j % 4 == 3 and nb < nnb:
                proj_sub(nb, "v")
            if j % 4 == 1:
                k = (j + 3) // 4
                if k <= nnb - 1:
                    trans_unit(k)
            if j == 0:
                trans_unit(0)

        head_loop(0, 0, fill_w0h0)

        # ---- remaining loops: AV of the previous (h, w) interleaves ----
        seq = [(w, h) for w in range(nw) for h in range(2)]
        tail_O = {}
        tail_rem = [0]

        def tail_av_mm(tO, ib, et, jj, hl):
            nc.tensor.matmul(
                tO[:, 0:DIM_HEAD + 1],
                lhsT=et[:, ib * 128:(ib + 1) * 128],
                rhs=V2[:, jj, hl, 0:DIM_HEAD + 1],
                start=(jj == 0), stop=(jj == nj - 1))

        def make_q_extra(nb):
            return lambda: proj_sub(nb, "q")

        extras = {1: make_q_extra(4), 2: make_q_extra(5),
                  3: make_q_extra(6), 4: make_q_extra(7)}
        for idx in range(1, len(seq)):
            w, h = seq[idx]
            wp, hp = seq[idx - 1]
            if w >= 1 and h == 0:
                # OTn(w-1) finishes early in this loop; start its outproj
                fill = make_av_fill(hp, wp, outproj_w=w - 1, op_base=0,
                                    op_cnt=4, op_j0_=op_j0h0,
                                    extra=extras.get(idx))
            elif w >= 1 and h == 1:
                fill = make_av_fill(hp, wp, outproj_w=w - 1, op_base=4,
                                    op_cnt=4, extra=extras.get(idx),
                                    tail_h=h if idx == len(seq) - 1 else None)
            else:
                fill = make_av_fill(hp, wp, extra=extras.get(idx))
            head_loop(w, h, fill)

        # ---- tail: AV of the last loop + final output projection ----
        wp, hp = seq[-1]
        ets = ETS.pop((wp, hp))
        while tail_rem[0] < nj:
            jj = tail_rem[0]
            for ib in range(2):
                tail_av_mm(tail_O[ib], ib, ets[jj], jj, hp)
            tail_rem[0] += 1
        for ib in range(2):
            o_finish(hp, wp, ib, tail_O.pop(ib))
        for ib in range(2, nib):
            O = psO.tile([128, 512], f32, name="O", tag="po")
            for jj in range(nj):
                nc.tensor.matmul(
                    O[:, 0:DIM_HEAD + 1],
                    lhsT=ets[jj][:, ib * 128:(ib + 1) * 128],
                    rhs=V2[:, jj, hp, 0:DIM_HEAD + 1],
                    start=(jj == 0), stop=(jj == nj - 1))
            o_finish(hp, wp, ib, O)
            if ib >= 3:
                otn_dma(wp, ib - 3)
            if ib == nib - 1:
                otn_dma(wp, nib - 3)
                for k in range(nwc * 2):
                    outproj_unit(nw - 1, k, tail=True)
        for ib in range(nib - 2, nib):
            otn_dma(wp, ib)
        for k in range(nwc * 2, nwc * 4):
            outproj_unit(nw - 1, k, tail=True)
        if dbg:
            nc.sync.dma_start(out=dbg_qT, in_=qT)
            nc.sync.dma_start(out=dbg_kT, in_=kT)
            nc.sync.dma_start(out=dbg_V2,
                              in_=V2.rearrange("p a b c -> p (a b c)"))
            nc.sync.dma_start(out=dbg_OTn, in_=OTn)

    with tile.TileContext(nc) as tc:
        with ExitStack() as ctx:
            emit_body(tc, ctx)

    nc.compile()
    return nc


def make_in_maps(x, Wq, Wk, Wv, Wo):
    x = np.asarray(x, np.float32)
    Wq = np.asarray(Wq, np.float32)
    Wk = np.asarray(Wk, np.float32)
    Wv = np.asarray(Wv, np.float32)
    Wo = np.asarray(Wo, np.float32)
    import ml_dtypes
    bf16 = ml_dtypes.bfloat16
    in_maps = []
    for c in range(NCORES):
        b, p = divmod(c, NCORES // B)
        e0 = p * E
        in_maps.append({
            "xT": np.ascontiguousarray(x[b].T).astype(bf16),
            "wqT": np.ascontiguousarray(Wq.T[:, e0:e0 + E]).astype(bf16),
            "wkT": np.ascontiguousarray(Wk.T[:, e0:e0 + E]).astype(bf16),
            "wvT": np.ascontiguousarray(Wv.T[:, e0:e0 + E]).astype(bf16),
            "woT": np.ascontiguousarray(Wo.T[e0:e0 + E, :]).astype(np.float16),
        })
    return in_maps


LAST_RESULTS = None


def kernel(x, Wq, Wk, Wv, Wo, bo):
    global LAST_RESULTS
    from concourse.bass_utils import run_bass_kernel_spmd

    if "nc" not in _CACHE:
        _CACHE["nc"] = build_program()
    nc = _CACHE["nc"]

    in_maps = make_in_maps(x, Wq, Wk, Wv, Wo)
    res = run_bass_kernel_spmd(nc, in_maps, core_ids=list(range(NCORES)))
    LAST_RESULTS = res

    y = np.zeros((B, N, D), np.float32)
    for c in range(NCORES):
        b = c // (NCORES // B)
        y[b] += res.results[c]["yT"].T
    y += np.asarray(bo, np.float32)
    return y
